# revision 24
# baseline (speedup 1.0000x reference)
"""Trainium2 Bass kernel for nn_CPWGenerator (B=16384, D=128, P=10, F=1024).

Data-parallel over batch across 8 NeuronCores (2048 rows/core). Per core:

  - x is host-cast to fp16 (same 11-bit mantissa as the f32r rounding the
    PE applies anyway) and loaded pre-transposed via the xbar DMA-transpose
    on the SP queue — no PE transposes, no PSUM round-trip for x.
  - feature-major 3-layer MLPs (control-point head + weight head), all
    operands fp16 (identical precision to f32r matmuls); relu/copy
    evacuations split between ACT and DVE to balance the two engines.
  - softmax denominator cancels: out = (G@(e*cpm)) / (G@e) with raw
    Gaussians G (row-normalization of the reference basis cancels in the
    ratio; the +1e-8 eps shifts the result by <1e-7 relative — measured).
  - The ratio num/den is evaluated on a COARSE t-grid of M=128 points
    (num/den are sums of sigma=0.1 Gaussians, so the ratio is smooth;
    linear interpolation back to F=1024 contributes <6e-4 relative error,
    measured against the reference on the real data distribution).
  - The pair-mean matrix P20 is folded into the coarse numerator matmul:
    GP = G @ P20^T, so numc = GP @ (cp * e2) needs no separate pairing
    matmul and no PSUM round-trip for the product.
  - The last W-MLP layer's weight rows are laid out [w3[q//2] x20; 0 x12;
    w3 x10] so ONE exp evacuation yields both the pair-aligned e (rows
    0..19) and the plain e (rows 32..41, base partition 32 for the matmul).
  - Interpolation to 1024 features is a constant fp16 matmul (K=128, full
    PE utilization) per 128-row batch tile; its PSUM tile is copied out as
    fp16 (ACT/DVE alternating) and DMA'd as fp16 (half the output bytes);
    the host upcasts to fp32.
  - Emission is software-pipelined: block b's 8 interp/evac/DMA units are
    interleaved with block b+1's MLP stages so ACT/DVE stay saturated.

Total added error ~1.1e-3 relative vs the 2e-2 budget.
"""
import sys
if "/opt/trn_rl_repo" not in sys.path:
    sys.path.insert(0, "/opt/trn_rl_repo")

from contextlib import ExitStack

import numpy as np

import concourse.bacc as bacc
import concourse.mybir as mybir
import concourse.tile as tile
from concourse.bass_utils import run_bass_kernel_spmd

F32 = mybir.dt.float32
F32R = mybir.dt.float32r
F16 = mybir.dt.float16
AF = mybir.ActivationFunctionType
ALU = mybir.AluOpType

# problem shapes (hardcoded per contest contract)
B, D, P, F = 16384, 128, 10, 1024
NCORES = 8
BC = B // NCORES          # rows per core = 2048
BLOCKS = [512, 512, 512, 512]  # batch blocks, processed in concurrent pairs
NBLK = len(BLOCKS)
M = 128                   # coarse t-grid points
EPS = 1e-8
SIG = 1.0 / P

# fp16 const blob column offsets (wh)
_C_W1T = 0            # [128 x 128]
_C_WW1T = 128         # [128 x 64]
_C_W2AT = 192         # [128 x 128]
_C_W2BT = 320         # [128 x 128]
_C_W3AT = 448         # [128 x 20]
_C_W3BT = 468         # [128 x 20]
_C_WW2T = 488         # [64  x 128]
_C_WW3T = 616         # [128 x 42] (q<20 -> w3[q//2]; 20..31 zero; 32+p -> w3[p])
_C_GCT = 658          # [10  x 128] gc^T at rows 32..41 (lhsT for denc)
_C_GPT = 786          # [20  x 128] (gc@P20^T)^T at rows 0..19 (lhsT for numc)
_C_I = 914            # [128 x 1024] interp matrix
C_H = 914 + F

# fp32 const blob columns (wf): per-partition bias vectors
_C_B1 = 0
_C_B2A = 1
_C_B2B = 2
_C_B3 = 3             # rows 0..19
_C_WB1 = 4            # rows 0..63
_C_WB2 = 5
_C_WB3 = 6            # rows 0..19 pair-dup, rows 32..41 plain
C_F = 7

# evac engine per (stage, block parity): spine A -> ACT, spine B -> DVE
EVAC_ENG = {("h1", 0): "act", ("g1", 0): "act", ("h2a", 0): "act",
            ("h2b", 0): "act", ("g2", 0): "act",
            ("h1", 1): "dve", ("g1", 1): "dve", ("h2a", 1): "dve",
            ("h2b", 1): "dve", ("g2", 1): "dve"}
# out-tile evac engine cycle, tunable
OUT_ENG = ["dve", "act", "act", "dve", "act", "dve", "act",
           "dve", "act", "act", "dve", "act", "dve", "act", "dve", "act"]
# how many MLP stage-units to emit between queued out-units
OUT_EVERY = 1


def round_f32r(x: np.ndarray) -> np.ndarray:
    """fp32 -> fp32r rounding (keep 11 explicit mantissa bits, RNE)."""
    u = np.ascontiguousarray(x, dtype=np.float32).view(np.uint32)
    keep = np.uint32(0xFFFFF000)
    half = np.uint32(0x800)
    lsb = (u >> np.uint32(12)) & np.uint32(1)
    r = (u + half - np.uint32(1) + lsb) & keep
    return r.view(np.float32)


def coarse_gaussians() -> np.ndarray:
    """Raw (unnormalized) Gaussian basis sampled on the coarse grid: [M, P]."""
    tc = np.linspace(0.0, 1.0, M, dtype=np.float64)
    c = (np.arange(P, dtype=np.float64) / (P - 1))
    g = np.exp(-((tc[:, None] - c[None, :]) ** 2) / (2.0 * SIG * SIG))
    return g.astype(np.float32)


def interp_matrix() -> np.ndarray:
    """Linear interpolation matrix I [M, F]: out[:, f] = sum_m rc[:, m]*I[m, f]."""
    t = np.linspace(0.0, 1.0, F, dtype=np.float64)
    pos = t * (M - 1)
    k = np.minimum(np.floor(pos).astype(np.int64), M - 2)
    a = (pos - k).astype(np.float32)
    I = np.zeros((M, F), np.float32)
    I[k, np.arange(F)] = 1.0 - a
    I[k + 1, np.arange(F)] = a
    return I


def build_program():
    nc = bacc.Bacc()
    x_in = nc.declare_dram_parameter("x", [BC, D], F16, isOutput=False)
    wh_in = nc.declare_dram_parameter("wh", [128, C_H], F16, isOutput=False)
    wf_in = nc.declare_dram_parameter("wf", [128, C_F], F32, isOutput=False)
    out = nc.declare_dram_parameter("out", [BC, F], F16, isOutput=True)

    with tile.TileContext(nc) as tc, ExitStack() as ctx:
        cpool = ctx.enter_context(tc.tile_pool(name="const", bufs=1))
        spool = ctx.enter_context(tc.tile_pool(name="work", bufs=4))
        tpool = ctx.enter_context(tc.tile_pool(name="tiny", bufs=4))
        rpool = ctx.enter_context(tc.tile_pool(name="ratio", bufs=4))
        opool = ctx.enter_context(tc.tile_pool(name="outp", bufs=4))
        mpool = ctx.enter_context(tc.tile_pool(name="psmlp", bufs=4, space="PSUM"))
        qpool = ctx.enter_context(tc.tile_pool(name="psout", bufs=2, space="PSUM"))

        wh = cpool.tile([128, C_H], F16)
        wf = cpool.tile([128, C_F], F32)
        xt = cpool.tile([128, BC], F16)   # feature-major x (d on partitions)

        # trigger the ACT table load immediately (off the critical spine)
        dum = cpool.tile([1, 16], F32)
        nc.vector.memset(dum[:], 0.0)
        nc.scalar.activation(dum[:], dum[:], AF.Relu)

        # per-block x-transposes + layer-1 weights on SP (gate the first
        # matmuls); biases and remaining consts on the gpsimd queue
        x0 = 0
        for nb_ in BLOCKS:
            nc.sync.dma_start_transpose(xt[:, x0:x0 + nb_],
                                        x_in[x0:x0 + nb_, :])
            x0 += nb_
        nc.gpsimd.dma_start(wh[:, 0:192], wh_in[:, 0:192])
        nc.gpsimd.dma_start(wf[:], wf_in[:])
        nc.gpsimd.dma_start(wh[:, 192:_C_I], wh_in[:, 192:_C_I])
        nc.gpsimd.dma_start(wh[:, _C_I:C_H], wh_in[:, _C_I:C_H])

        def mm(out_ap, lhsT, rhs, start=True, stop=True):
            # ISA caps a matmul's moving dim at 512 columns
            n = rhs.shape[-1]
            for c0 in range(0, n, 512):
                c1 = min(c0 + 512, n)
                nc.tensor.matmul(out_ap[:, c0:c1], lhsT, rhs[:, c0:c1],
                                 start=start, stop=stop)

        def evac_relu(stage, blk, dst, src, bias_ap):
            if EVAC_ENG[(stage, blk % 2)] == "act":
                nc.scalar.activation(dst, src, AF.Relu, bias=bias_ap)
            else:
                nc.vector.tensor_scalar(dst, src, bias_ap, 0.0,
                                        ALU.add, ALU.max)

        def mlp_units(blk):
            """Emission units (closures) for one block's MLP+coarse chain."""
            x0 = sum(BLOCKS[:blk])
            NB = BLOCKS[blk]
            xb = xt[:, x0:x0 + NB]
            st = {}

            def u_h1():
                st["h1p"] = mpool.tile([128, NB], F32, tag="mlp", name="h1p")
                mm(st["h1p"][:], wh[:, _C_W1T:_C_W1T + 128], xb)
                st["h1"] = spool.tile([128, NB], F16, tag="h1", name="h1")
                evac_relu("h1", blk, st["h1"][:], st["h1p"][:],
                          wf[:, _C_B1:_C_B1 + 1])

            def u_g1():
                st["g1p"] = mpool.tile([64, NB], F32, tag="mlp", name="g1p")
                mm(st["g1p"][:], wh[:, _C_WW1T:_C_WW1T + 64], xb)
                st["g1"] = spool.tile([64, NB], F16, tag="g1", name="g1")
                evac_relu("g1", blk, st["g1"][:], st["g1p"][:],
                          wf[0:64, _C_WB1:_C_WB1 + 1])

            def u_h2a():
                st["h2ap"] = mpool.tile([128, NB], F32, tag="mlp", name="h2ap")
                mm(st["h2ap"][:], wh[:, _C_W2AT:_C_W2AT + 128], st["h1"][:])
                st["h2a"] = spool.tile([128, NB], F16, tag="h2a", name="h2a")
                evac_relu("h2a", blk, st["h2a"][:], st["h2ap"][:],
                          wf[:, _C_B2A:_C_B2A + 1])

            def u_h2b():
                st["h2bp"] = mpool.tile([128, NB], F32, tag="mlp", name="h2bp")
                mm(st["h2bp"][:], wh[:, _C_W2BT:_C_W2BT + 128], st["h1"][:])
                st["h2b"] = spool.tile([128, NB], F16, tag="h2b", name="h2b")
                evac_relu("h2b", blk, st["h2b"][:], st["h2bp"][:],
                          wf[:, _C_B2B:_C_B2B + 1])

            def u_g2():
                st["g2p"] = mpool.tile([128, NB], F32, tag="mlp", name="g2p")
                mm(st["g2p"][:], wh[0:64, _C_WW2T:_C_WW2T + 128], st["g1"][:])
                st["g2"] = spool.tile([128, NB], F16, tag="g2", name="g2")
                evac_relu("g2", blk, st["g2"][:], st["g2p"][:],
                          wf[:, _C_WB2:_C_WB2 + 1])

            def u_cp():
                st["cpp"] = mpool.tile([20, NB], F32, tag="mlp", name="cpp")
                mm(st["cpp"][:], wh[:, _C_W3AT:_C_W3AT + 20], st["h2a"][:],
                   stop=False)
                mm(st["cpp"][:], wh[:, _C_W3BT:_C_W3BT + 20], st["h2b"][:],
                   start=False, stop=True)
                st["cp"] = tpool.tile([20, NB], F16, tag="cp", name="cp")
                nc.scalar.activation(st["cp"][:], st["cpp"][:], AF.Tanh,
                                     bias=wf[0:20, _C_B3:_C_B3 + 1])

            def u_e():
                st["wlp"] = mpool.tile([42, NB], F32, tag="mlp", name="wlp")
                mm(st["wlp"][:], wh[:, _C_WW3T:_C_WW3T + 42], st["g2"][:])
                st["e"] = tpool.tile([42, NB], F16, tag="e", name="e")
                nc.scalar.activation(st["e"][:], st["wlp"][:], AF.Exp,
                                     bias=wf[0:42, _C_WB3:_C_WB3 + 1])

            def u_v():
                st["v"] = tpool.tile([20, NB], F16, tag="v", name="v")
                nc.vector.tensor_mul(st["v"][:], st["cp"][:],
                                     st["e"][0:20, :])

            def u_den():
                st["dencp"] = mpool.tile([128, NB], F32, tag="mlp", name="dencp")
                mm(st["dencp"][:], wh[32:42, _C_GCT:_C_GCT + 128],
                   st["e"][32:42, :])
                st["r"] = rpool.tile([128, NB], F32, tag="r", name="r")
                nc.vector.reciprocal_approx_fast(out=st["r"][:],
                                                 in_=st["dencp"][:])

            def u_num():
                st["numcp"] = mpool.tile([128, NB], F32, tag="mlp", name="numcp")
                mm(st["numcp"][:], wh[0:20, _C_GPT:_C_GPT + 128], st["v"][:])
                st["rc"] = rpool.tile([128, NB], F16, tag="rc", name="rc")
                nc.vector.tensor_mul(st["rc"][:], st["numcp"][:], st["r"][:])

            return st, [u_h1, u_g1, u_h2a, u_h2b, u_g2, u_cp, u_e,
                        u_v, u_den, u_num]

        out_seq = [0]

        def out_unit(blk, st, j):
            def u():
                po = qpool.tile([128, F], F32, tag="out")
                mm(po[:], st["rc"][:, 128 * j:128 * (j + 1)],
                   wh[:, _C_I:_C_I + F])
                o16 = opool.tile([128, F], F16)
                eng = OUT_ENG[out_seq[0] % len(OUT_ENG)]
                out_seq[0] += 1
                if eng == "act":
                    nc.scalar.copy(o16[:], po[:])
                else:
                    nc.vector.tensor_copy(o16[:], po[:])
                r0 = sum(BLOCKS[:blk]) + 128 * j
                dma_eng = nc.sync if out_seq[0] % 2 else nc.gpsimd
                dma_eng.dma_start(out[r0:r0 + 128, :], o16[:])
            return u

        pending = []
        for pa in range(0, NBLK, 2):
            stA, unitsA = mlp_units(pa)
            stB, unitsB = mlp_units(pa + 1)
            tick = 0
            for uA, uB in zip(unitsA, unitsB):
                uA()
                uB()
                tick += 1
                if pending and tick % OUT_EVERY == 0:
                    pending.pop(0)()
                if pending and tick % OUT_EVERY == 0:
                    pending.pop(0)()
            pend_new = []
            for j in range(BLOCKS[pa] // 128):
                pend_new.append(out_unit(pa, stA, j))
                pend_new.append(out_unit(pa + 1, stB, j))
            pending.extend(pend_new)
        for u in pending:
            u()

    nc.compile()
    return nc


def host_consts(cp_w1, cp_b1, cp_w2, cp_b2, cp_w3, cp_b3,
                w_w1, w_b1, w_w2, w_b2, w_w3, w_b3):
    gc = coarse_gaussians()                    # [M, P]
    p20 = np.zeros((20, P), np.float32)
    for p in range(P):
        p20[2 * p, p] = 0.5
        p20[2 * p + 1, p] = 0.5
    gp = gc @ p20.T                            # [M, 20]

    wh = np.zeros((128, C_H), np.float16)
    wh[:, _C_W1T:_C_W1T + 128] = cp_w1.T.astype(np.float16)
    w2t = cp_w2.T.astype(np.float16)           # [128, 256]
    wh[:, _C_W2AT:_C_W2AT + 128] = w2t[:, 0:128]
    wh[:, _C_W2BT:_C_W2BT + 128] = w2t[:, 128:256]
    w3t = cp_w3.T.astype(np.float16)           # [256, 20]
    wh[:, _C_W3AT:_C_W3AT + 20] = w3t[0:128]
    wh[:, _C_W3BT:_C_W3BT + 20] = w3t[128:256]
    wh[:, _C_WW1T:_C_WW1T + 64] = w_w1.T.astype(np.float16)
    wh[0:64, _C_WW2T:_C_WW2T + 128] = w_w2.T.astype(np.float16)
    w3w = w_w3.T.astype(np.float16)            # [128, 10]
    wh[:, _C_WW3T:_C_WW3T + 20] = np.repeat(w3w, 2, axis=1)
    wh[:, _C_WW3T + 32:_C_WW3T + 42] = w3w
    wh[32:42, _C_GCT:_C_GCT + 128] = gc.T.astype(np.float16)
    wh[0:20, _C_GPT:_C_GPT + 128] = gp.T.astype(np.float16)
    wh[:, _C_I:_C_I + F] = interp_matrix().astype(np.float16)

    wf = np.zeros((128, C_F), np.float32)
    wf[:, _C_B1] = cp_b1
    wf[:, _C_B2A] = cp_b2[0:128]
    wf[:, _C_B2B] = cp_b2[128:256]
    wf[0:20, _C_B3] = cp_b3
    wf[0:64, _C_WB1] = w_b1
    wf[:, _C_WB2] = w_b2
    wf[0:20, _C_WB3] = np.repeat(w_b3, 2)
    wf[32:42, _C_WB3] = w_b3
    return wh, wf


_NC_CACHE = None


def get_program():
    global _NC_CACHE
    if _NC_CACHE is None:
        _NC_CACHE = build_program()
    return _NC_CACHE


def kernel(x, cp_w1, cp_b1, cp_w2, cp_b2, cp_w3, cp_b3,
           w_w1, w_b1, w_w2, w_b2, w_w3, w_b3, _return_raw=False):
    x16 = np.asarray(x, np.float32).astype(np.float16)
    wh, wf = host_consts(
        np.asarray(cp_w1, np.float32), np.asarray(cp_b1, np.float32),
        np.asarray(cp_w2, np.float32), np.asarray(cp_b2, np.float32),
        np.asarray(cp_w3, np.float32), np.asarray(cp_b3, np.float32),
        np.asarray(w_w1, np.float32), np.asarray(w_b1, np.float32),
        np.asarray(w_w2, np.float32), np.asarray(w_b2, np.float32),
        np.asarray(w_w3, np.float32), np.asarray(w_b3, np.float32))

    nc = get_program()
    in_maps = [
        {"x": np.ascontiguousarray(x16[i * BC:(i + 1) * BC]),
         "wh": wh, "wf": wf}
        for i in range(NCORES)
    ]
    res = run_bass_kernel_spmd(nc, in_maps, list(range(NCORES)))
    outs = [res.results[i]["out"] for i in range(NCORES)]
    full = np.concatenate(outs, axis=0).astype(np.float32)
    if _return_raw:
        return full, res
    return full


# revision 25
# speedup vs baseline: 1.0012x; 1.0012x over previous
"""Trainium2 Bass kernel for nn_CPWGenerator (B=16384, D=128, P=10, F=1024).

Data-parallel over batch across 8 NeuronCores (2048 rows/core). Per core:

  - x is host-cast to fp16 (same 11-bit mantissa as the f32r rounding the
    PE applies anyway) and loaded pre-transposed via the xbar DMA-transpose
    on the SP queue — no PE transposes, no PSUM round-trip for x.
  - feature-major 3-layer MLPs (control-point head + weight head), all
    operands fp16 (identical precision to f32r matmuls); relu/copy
    evacuations split between ACT and DVE to balance the two engines.
  - softmax denominator cancels: out = (G@(e*cpm)) / (G@e) with raw
    Gaussians G (row-normalization of the reference basis cancels in the
    ratio; the +1e-8 eps shifts the result by <1e-7 relative — measured).
  - The ratio num/den is evaluated on a COARSE t-grid of M=128 points
    (num/den are sums of sigma=0.1 Gaussians, so the ratio is smooth;
    linear interpolation back to F=1024 contributes <6e-4 relative error,
    measured against the reference on the real data distribution).
  - The pair-mean matrix P20 is folded into the coarse numerator matmul:
    GP = G @ P20^T, so numc = GP @ (cp * e2) needs no separate pairing
    matmul and no PSUM round-trip for the product.
  - The last W-MLP layer's weight rows are laid out [w3[q//2] x20; 0 x12;
    w3 x10] so ONE exp evacuation yields both the pair-aligned e (rows
    0..19) and the plain e (rows 32..41, base partition 32 for the matmul).
  - Interpolation to 1024 features is a constant fp16 matmul (K=128, full
    PE utilization) per 128-row batch tile; its PSUM tile is copied out as
    fp16 (ACT/DVE alternating) and DMA'd as fp16 (half the output bytes);
    the host upcasts to fp32.
  - Emission is software-pipelined: block b's 8 interp/evac/DMA units are
    interleaved with block b+1's MLP stages so ACT/DVE stay saturated.

Total added error ~1.1e-3 relative vs the 2e-2 budget.
"""
import sys
if "/opt/trn_rl_repo" not in sys.path:
    sys.path.insert(0, "/opt/trn_rl_repo")

from contextlib import ExitStack

import numpy as np

import concourse.bacc as bacc
import concourse.mybir as mybir
import concourse.tile as tile
from concourse.bass_utils import run_bass_kernel_spmd

F32 = mybir.dt.float32
F32R = mybir.dt.float32r
F16 = mybir.dt.float16
AF = mybir.ActivationFunctionType
ALU = mybir.AluOpType

# problem shapes (hardcoded per contest contract)
B, D, P, F = 16384, 128, 10, 1024
NCORES = 8
BC = B // NCORES          # rows per core = 2048
BLOCKS = [512, 512, 512, 512]  # batch blocks, processed in concurrent pairs
NBLK = len(BLOCKS)
M = 128                   # coarse t-grid points
EPS = 1e-8
SIG = 1.0 / P

# fp16 const blob column offsets (wh)
_C_W1T = 0            # [128 x 128]
_C_WW1T = 128         # [128 x 64]
_C_W2AT = 192         # [128 x 128]
_C_W2BT = 320         # [128 x 128]
_C_W3AT = 448         # [128 x 20]
_C_W3BT = 468         # [128 x 20]
_C_WW2T = 488         # [64  x 128]
_C_WW3T = 616         # [128 x 42] (q<20 -> w3[q//2]; 20..31 zero; 32+p -> w3[p])
_C_GCT = 658          # [10  x 128] gc^T at rows 32..41 (lhsT for denc)
_C_GPT = 786          # [20  x 128] (gc@P20^T)^T at rows 0..19 (lhsT for numc)
_C_I = 914            # [128 x 1024] interp matrix
C_H = 914 + F

# fp32 const blob columns (wf): per-partition bias vectors
_C_B1 = 0
_C_B2A = 1
_C_B2B = 2
_C_B3 = 3             # rows 0..19
_C_WB1 = 4            # rows 0..63
_C_WB2 = 5
_C_WB3 = 6            # rows 0..19 pair-dup, rows 32..41 plain
C_F = 7

# evac engine per (stage, block parity): spine A -> ACT, spine B -> DVE
EVAC_ENG = {("h1", 0): "act", ("g1", 0): "act", ("h2a", 0): "act",
            ("h2b", 0): "act", ("g2", 0): "act",
            ("h1", 1): "dve", ("g1", 1): "dve", ("h2a", 1): "dve",
            ("h2b", 1): "dve", ("g2", 1): "dve"}
# out-tile evac engine cycle, tunable
OUT_ENG = ["dve", "act", "act", "dve", "act", "dve", "act",
           "dve", "act", "act", "dve", "act", "dve", "act", "dve", "act"]
# how many MLP stage-units to emit between queued out-units
OUT_EVERY = 1


def round_f32r(x: np.ndarray) -> np.ndarray:
    """fp32 -> fp32r rounding (keep 11 explicit mantissa bits, RNE)."""
    u = np.ascontiguousarray(x, dtype=np.float32).view(np.uint32)
    keep = np.uint32(0xFFFFF000)
    half = np.uint32(0x800)
    lsb = (u >> np.uint32(12)) & np.uint32(1)
    r = (u + half - np.uint32(1) + lsb) & keep
    return r.view(np.float32)


def coarse_gaussians() -> np.ndarray:
    """Raw (unnormalized) Gaussian basis sampled on the coarse grid: [M, P]."""
    tc = np.linspace(0.0, 1.0, M, dtype=np.float64)
    c = (np.arange(P, dtype=np.float64) / (P - 1))
    g = np.exp(-((tc[:, None] - c[None, :]) ** 2) / (2.0 * SIG * SIG))
    return g.astype(np.float32)


def interp_matrix() -> np.ndarray:
    """Linear interpolation matrix I [M, F]: out[:, f] = sum_m rc[:, m]*I[m, f]."""
    t = np.linspace(0.0, 1.0, F, dtype=np.float64)
    pos = t * (M - 1)
    k = np.minimum(np.floor(pos).astype(np.int64), M - 2)
    a = (pos - k).astype(np.float32)
    I = np.zeros((M, F), np.float32)
    I[k, np.arange(F)] = 1.0 - a
    I[k + 1, np.arange(F)] = a
    return I


def build_program():
    nc = bacc.Bacc()
    x_in = nc.declare_dram_parameter("x", [BC, D], F16, isOutput=False)
    wh_in = nc.declare_dram_parameter("wh", [128, C_H], F16, isOutput=False)
    wf_in = nc.declare_dram_parameter("wf", [128, C_F], F32, isOutput=False)
    out = nc.declare_dram_parameter("out", [BC, F], F16, isOutput=True)

    with tile.TileContext(nc) as tc, ExitStack() as ctx:
        cpool = ctx.enter_context(tc.tile_pool(name="const", bufs=1))
        spool = ctx.enter_context(tc.tile_pool(name="work", bufs=4))
        tpool = ctx.enter_context(tc.tile_pool(name="tiny", bufs=4))
        rpool = ctx.enter_context(tc.tile_pool(name="ratio", bufs=4))
        opool = ctx.enter_context(tc.tile_pool(name="outp", bufs=4))
        mpool = ctx.enter_context(tc.tile_pool(name="psmlp", bufs=4, space="PSUM"))
        qpool = ctx.enter_context(tc.tile_pool(name="psout", bufs=2, space="PSUM"))

        wh = cpool.tile([128, C_H], F16)
        wf = cpool.tile([128, C_F], F32)
        xt = cpool.tile([128, BC], F16)   # feature-major x (d on partitions)

        # trigger the ACT table load immediately (off the critical spine)
        dum = cpool.tile([1, 16], F32)
        nc.vector.memset(dum[:], 0.0)
        nc.scalar.activation(dum[:], dum[:], AF.Relu)

        # per-block x-transposes + layer-1 weights on SP (gate the first
        # matmuls); biases and remaining consts on the gpsimd queue
        x0 = 0
        for nb_ in BLOCKS:
            nc.sync.dma_start_transpose(xt[:, x0:x0 + nb_],
                                        x_in[x0:x0 + nb_, :])
            x0 += nb_
        nc.gpsimd.dma_start(wh[:, 0:192], wh_in[:, 0:192])
        nc.gpsimd.dma_start(wf[:], wf_in[:])
        nc.gpsimd.dma_start(wh[:, 192:_C_I], wh_in[:, 192:_C_I])
        nc.gpsimd.dma_start(wh[:, _C_I:C_H], wh_in[:, _C_I:C_H])

        def mm(out_ap, lhsT, rhs, start=True, stop=True):
            # ISA caps a matmul's moving dim at 512 columns
            n = rhs.shape[-1]
            for c0 in range(0, n, 512):
                c1 = min(c0 + 512, n)
                nc.tensor.matmul(out_ap[:, c0:c1], lhsT, rhs[:, c0:c1],
                                 start=start, stop=stop)

        def evac_relu(stage, blk, dst, src, bias_ap):
            if EVAC_ENG[(stage, blk % 2)] == "act":
                nc.scalar.activation(dst, src, AF.Relu, bias=bias_ap)
            else:
                nc.vector.tensor_scalar(dst, src, bias_ap, 0.0,
                                        ALU.add, ALU.max)

        def mlp_units(blk):
            """Emission units (closures) for one block's MLP+coarse chain."""
            x0 = sum(BLOCKS[:blk])
            NB = BLOCKS[blk]
            xb = xt[:, x0:x0 + NB]
            st = {}

            def u_h1():
                st["h1p"] = mpool.tile([128, NB], F32, tag="mlp", name="h1p")
                mm(st["h1p"][:], wh[:, _C_W1T:_C_W1T + 128], xb)
                st["h1"] = spool.tile([128, NB], F16, tag="h1", name="h1")
                evac_relu("h1", blk, st["h1"][:], st["h1p"][:],
                          wf[:, _C_B1:_C_B1 + 1])

            def u_g1():
                st["g1p"] = mpool.tile([64, NB], F32, tag="mlp", name="g1p")
                mm(st["g1p"][:], wh[:, _C_WW1T:_C_WW1T + 64], xb)
                st["g1"] = spool.tile([64, NB], F16, tag="g1", name="g1")
                evac_relu("g1", blk, st["g1"][:], st["g1p"][:],
                          wf[0:64, _C_WB1:_C_WB1 + 1])

            def u_h2a():
                st["h2ap"] = mpool.tile([128, NB], F32, tag="mlp", name="h2ap")
                mm(st["h2ap"][:], wh[:, _C_W2AT:_C_W2AT + 128], st["h1"][:])
                st["h2a"] = spool.tile([128, NB], F16, tag="h2a", name="h2a")
                evac_relu("h2a", blk, st["h2a"][:], st["h2ap"][:],
                          wf[:, _C_B2A:_C_B2A + 1])

            def u_h2b():
                st["h2bp"] = mpool.tile([128, NB], F32, tag="mlp", name="h2bp")
                mm(st["h2bp"][:], wh[:, _C_W2BT:_C_W2BT + 128], st["h1"][:])
                st["h2b"] = spool.tile([128, NB], F16, tag="h2b", name="h2b")
                evac_relu("h2b", blk, st["h2b"][:], st["h2bp"][:],
                          wf[:, _C_B2B:_C_B2B + 1])

            def u_g2():
                st["g2p"] = mpool.tile([128, NB], F32, tag="mlp", name="g2p")
                mm(st["g2p"][:], wh[0:64, _C_WW2T:_C_WW2T + 128], st["g1"][:])
                st["g2"] = spool.tile([128, NB], F16, tag="g2", name="g2")
                evac_relu("g2", blk, st["g2"][:], st["g2p"][:],
                          wf[:, _C_WB2:_C_WB2 + 1])

            def u_cp():
                st["cpp"] = mpool.tile([20, NB], F32, tag="mlp", name="cpp")
                mm(st["cpp"][:], wh[:, _C_W3AT:_C_W3AT + 20], st["h2a"][:],
                   stop=False)
                mm(st["cpp"][:], wh[:, _C_W3BT:_C_W3BT + 20], st["h2b"][:],
                   start=False, stop=True)
                st["cp"] = tpool.tile([20, NB], F16, tag="cp", name="cp")
                nc.scalar.activation(st["cp"][:], st["cpp"][:], AF.Tanh,
                                     bias=wf[0:20, _C_B3:_C_B3 + 1])

            def u_e():
                st["wlp"] = mpool.tile([42, NB], F32, tag="mlp", name="wlp")
                mm(st["wlp"][:], wh[:, _C_WW3T:_C_WW3T + 42], st["g2"][:])
                st["e"] = tpool.tile([42, NB], F16, tag="e", name="e")
                nc.scalar.activation(st["e"][:], st["wlp"][:], AF.Exp,
                                     bias=wf[0:42, _C_WB3:_C_WB3 + 1])

            def u_v():
                st["v"] = tpool.tile([20, NB], F16, tag="v", name="v")
                nc.vector.tensor_mul(st["v"][:], st["cp"][:],
                                     st["e"][0:20, :])

            def u_den():
                st["dencp"] = mpool.tile([128, NB], F32, tag="mlp", name="dencp")
                mm(st["dencp"][:], wh[32:42, _C_GCT:_C_GCT + 128],
                   st["e"][32:42, :])
                st["r"] = rpool.tile([128, NB], F32, tag="r", name="r")
                nc.vector.reciprocal_approx_fast(out=st["r"][:],
                                                 in_=st["dencp"][:])

            def u_num():
                st["numcp"] = mpool.tile([128, NB], F32, tag="mlp", name="numcp")
                mm(st["numcp"][:], wh[0:20, _C_GPT:_C_GPT + 128], st["v"][:])
                st["rc"] = rpool.tile([128, NB], F16, tag="rc", name="rc")
                nc.vector.tensor_mul(st["rc"][:], st["numcp"][:], st["r"][:])

            return st, [u_h1, u_g1, u_h2a, u_h2b, u_g2, u_cp, u_e,
                        u_v, u_den, u_num]

        out_seq = [0]

        def out_unit(blk, st, j):
            def u():
                po = qpool.tile([128, F], F32, tag="out")
                mm(po[:], st["rc"][:, 128 * j:128 * (j + 1)],
                   wh[:, _C_I:_C_I + F])
                o16 = opool.tile([128, F], F16)
                eng = OUT_ENG[out_seq[0] % len(OUT_ENG)]
                out_seq[0] += 1
                if eng == "act":
                    nc.scalar.copy(o16[:], po[:])
                else:
                    nc.vector.tensor_copy(o16[:], po[:])
                r0 = sum(BLOCKS[:blk]) + 128 * j
                nc.sync.dma_start(out[r0:r0 + 128, :], o16[:])
            return u

        pending = []
        for pa in range(0, NBLK, 2):
            stA, unitsA = mlp_units(pa)
            stB, unitsB = mlp_units(pa + 1)
            tick = 0
            for uA, uB in zip(unitsA, unitsB):
                uA()
                uB()
                tick += 1
                if pending and tick % OUT_EVERY == 0:
                    pending.pop(0)()
                if pending and tick % OUT_EVERY == 0:
                    pending.pop(0)()
            pend_new = []
            for j in range(BLOCKS[pa] // 128):
                pend_new.append(out_unit(pa, stA, j))
                pend_new.append(out_unit(pa + 1, stB, j))
            pending.extend(pend_new)
        for u in pending:
            u()

    nc.compile()
    return nc


def host_consts(cp_w1, cp_b1, cp_w2, cp_b2, cp_w3, cp_b3,
                w_w1, w_b1, w_w2, w_b2, w_w3, w_b3):
    gc = coarse_gaussians()                    # [M, P]
    p20 = np.zeros((20, P), np.float32)
    for p in range(P):
        p20[2 * p, p] = 0.5
        p20[2 * p + 1, p] = 0.5
    gp = gc @ p20.T                            # [M, 20]

    wh = np.zeros((128, C_H), np.float16)
    wh[:, _C_W1T:_C_W1T + 128] = cp_w1.T.astype(np.float16)
    w2t = cp_w2.T.astype(np.float16)           # [128, 256]
    wh[:, _C_W2AT:_C_W2AT + 128] = w2t[:, 0:128]
    wh[:, _C_W2BT:_C_W2BT + 128] = w2t[:, 128:256]
    w3t = cp_w3.T.astype(np.float16)           # [256, 20]
    wh[:, _C_W3AT:_C_W3AT + 20] = w3t[0:128]
    wh[:, _C_W3BT:_C_W3BT + 20] = w3t[128:256]
    wh[:, _C_WW1T:_C_WW1T + 64] = w_w1.T.astype(np.float16)
    wh[0:64, _C_WW2T:_C_WW2T + 128] = w_w2.T.astype(np.float16)
    w3w = w_w3.T.astype(np.float16)            # [128, 10]
    wh[:, _C_WW3T:_C_WW3T + 20] = np.repeat(w3w, 2, axis=1)
    wh[:, _C_WW3T + 32:_C_WW3T + 42] = w3w
    wh[32:42, _C_GCT:_C_GCT + 128] = gc.T.astype(np.float16)
    wh[0:20, _C_GPT:_C_GPT + 128] = gp.T.astype(np.float16)
    wh[:, _C_I:_C_I + F] = interp_matrix().astype(np.float16)

    wf = np.zeros((128, C_F), np.float32)
    wf[:, _C_B1] = cp_b1
    wf[:, _C_B2A] = cp_b2[0:128]
    wf[:, _C_B2B] = cp_b2[128:256]
    wf[0:20, _C_B3] = cp_b3
    wf[0:64, _C_WB1] = w_b1
    wf[:, _C_WB2] = w_b2
    wf[0:20, _C_WB3] = np.repeat(w_b3, 2)
    wf[32:42, _C_WB3] = w_b3
    return wh, wf


_NC_CACHE = None


def get_program():
    global _NC_CACHE
    if _NC_CACHE is None:
        _NC_CACHE = build_program()
    return _NC_CACHE


def kernel(x, cp_w1, cp_b1, cp_w2, cp_b2, cp_w3, cp_b3,
           w_w1, w_b1, w_w2, w_b2, w_w3, w_b3, _return_raw=False):
    x16 = np.asarray(x, np.float32).astype(np.float16)
    wh, wf = host_consts(
        np.asarray(cp_w1, np.float32), np.asarray(cp_b1, np.float32),
        np.asarray(cp_w2, np.float32), np.asarray(cp_b2, np.float32),
        np.asarray(cp_w3, np.float32), np.asarray(cp_b3, np.float32),
        np.asarray(w_w1, np.float32), np.asarray(w_b1, np.float32),
        np.asarray(w_w2, np.float32), np.asarray(w_b2, np.float32),
        np.asarray(w_w3, np.float32), np.asarray(w_b3, np.float32))

    nc = get_program()
    in_maps = [
        {"x": np.ascontiguousarray(x16[i * BC:(i + 1) * BC]),
         "wh": wh, "wf": wf}
        for i in range(NCORES)
    ]
    res = run_bass_kernel_spmd(nc, in_maps, list(range(NCORES)))
    outs = [res.results[i]["out"] for i in range(NCORES)]
    full = np.concatenate(outs, axis=0).astype(np.float32)
    if _return_raw:
        return full, res
    return full
